# revision 2
# baseline (speedup 1.0000x reference)
"""BitLinear-1.58 forward on 8 trn2 NeuronCores.

out = x @ qw.T + bias, qw = clip(round(w / (eps + mean|w|)), -1, 1).

Strategy:
  - Quantize the weight on host with jnp (bit-identical to the reference's
    quantization, same jax backend), transpose to [in, out] and cast to bf16
    (ternary values are exact in bf16).
  - Cast/transpose x to [in, tok] bf16 on host.
  - Column-parallel across 8 cores: each core computes the full-token output
    for a 1024-wide slice of out_features with a Bass/Tile PE matmul
    (bf16 inputs, fp32 PSUM accumulation).
  - Concatenate the 8 output slices.
"""

import numpy as np
import ml_dtypes

B, S, IN, OUT = 4, 2048, 2048, 8192
N_CORES = 8
TOK = B * S
N_SHARD = OUT // N_CORES
SCALE_EPS = 1e-05

_CACHED_NC = None


def _build_nc():
    import concourse.mybir as mybir
    import concourse.tile as tile
    from concourse import bacc
    from concourse.kernels.tile_matmul import matmul_tile_kernel

    nc = bacc.Bacc(None, target_bir_lowering=False)

    x_t = nc.dram_tensor("x_t", [IN, TOK], mybir.dt.bfloat16, kind="ExternalInput")
    w_t = nc.dram_tensor("w_t", [IN, N_SHARD], mybir.dt.bfloat16, kind="ExternalInput")
    out = nc.dram_tensor("out", [TOK, N_SHARD], mybir.dt.float32, kind="ExternalOutput")

    with tile.TileContext(nc) as tc:
        matmul_tile_kernel(
            tc,
            x_t[:, :],
            w_t[:, :],
            out[:, :],
            MAX_K_TILE_SIZE=2048,
        )

    nc.compile()
    return nc


def _get_nc():
    global _CACHED_NC
    if _CACHED_NC is None:
        _CACHED_NC = _build_nc()
    return _CACHED_NC


def _quantize_weight(weight: np.ndarray) -> np.ndarray:
    """Ternarize exactly as the reference does (same jax ops, same backend)."""
    import jax.numpy as jnp

    w = jnp.asarray(weight)
    scale = SCALE_EPS + jnp.mean(jnp.abs(w))
    quant = jnp.clip(jnp.round(w / scale), -1.0, 1.0)
    return np.asarray(quant, dtype=np.float32)


def _prepare_in_maps(x: np.ndarray, weight: np.ndarray):
    qw = _quantize_weight(weight)  # [OUT, IN] ternary fp32

    # [IN, OUT] bf16 (exact: values are -1/0/1)
    w_t = np.ascontiguousarray(qw.T).astype(ml_dtypes.bfloat16)
    # [IN, TOK] bf16
    x_t = np.ascontiguousarray(x.reshape(TOK, IN).T).astype(ml_dtypes.bfloat16)

    return [
        {
            "x_t": x_t,
            "w_t": np.ascontiguousarray(w_t[:, i * N_SHARD : (i + 1) * N_SHARD]),
        }
        for i in range(N_CORES)
    ]


def _postprocess(outs: list, bias: np.ndarray) -> np.ndarray:
    out = np.concatenate([np.asarray(o) for o in outs], axis=1)  # [TOK, OUT] f32
    out = out.reshape(B, S, OUT)
    if np.any(bias):
        out = out + bias.astype(np.float32)
    return out


def kernel(x: np.ndarray, weight: np.ndarray, bias: np.ndarray) -> np.ndarray:
    from concourse.bass_utils import run_bass_kernel_spmd

    in_maps = _prepare_in_maps(x, weight)
    nc = _get_nc()
    res = run_bass_kernel_spmd(nc, in_maps, core_ids=list(range(N_CORES)))
    return _postprocess([r["out"] for r in res.results], bias)


# revision 6
# speedup vs baseline: 16.3671x; 16.3671x over previous
"""BitLinear-1.58 forward on 8 trn2 NeuronCores.

out = x @ qw.T + bias, qw = clip(round(w / (eps + mean|w|)), -1, 1).

Strategy:
  - Quantize the weight on host with jnp (bit-identical to the reference's
    quantization, same jax backend), transpose to [in, out] and cast to bf16
    (ternary values are exact in bf16).
  - Cast/transpose x to [in, tok] bf16 on host.
  - Column-parallel across 8 cores: each core computes the full-token output
    for a 1024-wide slice of out_features with a Bass/Tile PE matmul
    (bf16 inputs, fp32 PSUM accumulation).
  - Concatenate the 8 output slices.
"""

import numpy as np
import ml_dtypes

B, S, IN, OUT = 4, 2048, 2048, 8192
N_CORES = 8
TOK = B * S
N_SHARD = OUT // N_CORES
SCALE_EPS = 1e-05

_CACHED_NC = None


def _build_nc():
    import concourse.mybir as mybir
    import concourse.tile as tile
    from concourse import bacc
    from concourse.kernels.tile_matmul import matmul_tile_kernel

    nc = bacc.Bacc(None, target_bir_lowering=False)

    x_t = nc.dram_tensor("x_t", [IN, TOK], mybir.dt.bfloat16, kind="ExternalInput")
    w_t = nc.dram_tensor("w_t", [IN, N_SHARD], mybir.dt.bfloat16, kind="ExternalInput")
    out = nc.dram_tensor("out", [TOK, N_SHARD], mybir.dt.float32, kind="ExternalOutput")

    with tile.TileContext(nc) as tc:
        # PE warm-up: dummy matmuls with no data deps run while the first
        # input tiles are still DMA-ing in, so the HAM clock gate is already
        # released (2.4 GHz) when the real matmul stream starts.
        with (
            tc.tile_pool(name="warm", bufs=1) as warm_pool,
            tc.tile_pool(name="warm_psum", bufs=1, space="PSUM") as warm_psum,
        ):
            wl = warm_pool.tile([128, 512], mybir.dt.bfloat16)
            wp = warm_psum.tile([128, 512], mybir.dt.float32)
            nc.vector.memset(wl[:], 0.0)
            n_warm = 14
            for i in range(n_warm):
                nc.tensor.matmul(
                    wp[:], wl[:, :128], wl[:], start=(i == 0), stop=(i == n_warm - 1)
                )

        matmul_tile_kernel(
            tc,
            x_t[:, :],
            w_t[:, :],
            out[:, :],
            MAX_K_TILE_SIZE=256,
        )

    nc.compile()
    return nc


def _get_nc():
    global _CACHED_NC
    if _CACHED_NC is None:
        _CACHED_NC = _build_nc()
    return _CACHED_NC


def _quantize_weight(weight: np.ndarray) -> np.ndarray:
    """Ternarize exactly as the reference does (same jax ops, same backend)."""
    import jax.numpy as jnp

    w = jnp.asarray(weight)
    scale = SCALE_EPS + jnp.mean(jnp.abs(w))
    quant = jnp.clip(jnp.round(w / scale), -1.0, 1.0)
    return np.asarray(quant, dtype=np.float32)


def _prepare_in_maps(x: np.ndarray, weight: np.ndarray):
    qw = _quantize_weight(weight)  # [OUT, IN] ternary fp32

    # [IN, OUT] bf16 (exact: values are -1/0/1)
    w_t = np.ascontiguousarray(qw.T).astype(ml_dtypes.bfloat16)
    # [IN, TOK] bf16
    x_t = np.ascontiguousarray(x.reshape(TOK, IN).T).astype(ml_dtypes.bfloat16)

    return [
        {
            "x_t": x_t,
            "w_t": np.ascontiguousarray(w_t[:, i * N_SHARD : (i + 1) * N_SHARD]),
        }
        for i in range(N_CORES)
    ]


def _postprocess(outs: list, bias: np.ndarray) -> np.ndarray:
    out = np.concatenate([np.asarray(o) for o in outs], axis=1)  # [TOK, OUT] f32
    out = out.reshape(B, S, OUT)
    if np.any(bias):
        out = out + bias.astype(np.float32)
    return out


def kernel(x: np.ndarray, weight: np.ndarray, bias: np.ndarray) -> np.ndarray:
    from concourse.bass_utils import run_bass_kernel_spmd

    in_maps = _prepare_in_maps(x, weight)
    nc = _get_nc()
    res = run_bass_kernel_spmd(nc, in_maps, core_ids=list(range(N_CORES)))
    return _postprocess([r["out"] for r in res.results], bias)
